# revision 19
# baseline (speedup 1.0000x reference)
"""SE(3) attention block (GNN message passing) on 8 Trainium2 NeuronCores.

Strategy
--------
Edges are sorted by destination node on the host. Nodes are cut into tiles of
(<=128 nodes, <=2048 edge slots); each node's edge run is padded to an even
number of slots so slot PAIRS always share one dst node. Tiles are distributed
contiguously across the 8 cores, so every (node, head) softmax group lives
entirely on one core and inside one tile -> no cross-device collectives.

The destination-node query is shipped ONCE PER SLOT-PAIR (qrun2, [128 feat,
1024 pairs] per tile) instead of once per slot, halving what used to be the
largest input stream. On device the pair query is consumed directly by the
k*q product through a stride-0 broadcast access pattern: edge slots are
relabeled so all even slots occupy columns 0..1023 and odd slots 1024..2047
(host-side permutation, free), which turns the pair broadcast into a
middle-dim broadcast [128, 2, 1024] whose last dim stays packed -> DVE keeps
its 2x (16-bit) throughput for the product.

Per node tile the device kernel:
  1. widens the per-block dst map to a dense [128, 2048] map (one ACT op)
     and builds the one-hot edge->local-node matrix with a single 2x DVE
     is_equal against a premade iota,
  2. prodT = kT * q_pair broadcast (one 2x DVE op),
  3. per-head scores via 16 head-mask matmuls (N=8) into one PSUM bank,
  4. one exp over the tile's [128, 128] scores (ACT, bf16 out),
  5. widens ex to the interleaved 17-stride (ACT) and forms the
     [ex | ex*v] scatter rhs with one whole-tile DVE 2x multiply
     (v is sent from the host with a 1.0 column per head: 17 cols/head),
  6. 16 back-to-back scatter-add matmuls (bf16, N=136) accumulate into a
     [128, 136] PSUM tile.
The tile is then normalized by 1/sum(exp) (strided APs pull ssum/agg out of
the interleaved accumulator) and written out in bf16. The host scatters
per-tile rows back into the full [N, 32, 4] f32 output. GPSIMD is left idle:
TRN2's Pool engine rejects general tensor ops (ISA check) and its custom
gather library costs ~29us fixed per invocation.
"""

import math
import numpy as np

# ---------------------------------------------------------------- constants
N_CORES = 8
P = 128                 # partitions / nodes per tile / edges per block
F_BLOCKS = 16           # edge blocks per node tile
EPT = F_BLOCKS * P      # edge slots per tile (2048)
NPAIR = EPT // 2
T_PC = 51               # node tiles per core (multiple of OPACK)
OPACK = 3               # output tiles packed per DMA
H = 8                   # heads
NF = 128                # features per edge (32*4)
HS = NF // H            # head size (16)
HS1 = HS + 1            # interleaved head stride (ex + 16 features)
N_NODES = 50000
E_EDGES = 800000
PAD_DST = 300.0         # local-dst sentinel for padding edge slots
INV_SQRT_NF = 1.0 / math.sqrt(NF)

_CACHE = {}
LAST_RESULTS = None     # BassKernelResults of the most recent run (for test.py)


# ---------------------------------------------------------------- device IR
def build_nc(tpc=T_PC, f_blocks=F_BLOCKS, v_bf16=True):
    """Build the per-core Bass/Tile program (identical on all 8 cores)."""
    from contextlib import ExitStack

    import concourse.bacc as bacc
    import concourse.mybir as mybir
    from concourse.tile import TileContext

    f32 = mybir.dt.float32
    bf16 = mybir.dt.bfloat16
    vdt = bf16 if v_bf16 else f32
    ept = f_blocks * P
    npair = ept // 2

    vcols = f_blocks * (H * HS1 + 1)          # v17 + dl columns (2192)
    kvq_cols = ept + npair + vcols            # kt | qr | v17+dl

    nc = bacc.Bacc("TRN2", target_bir_lowering=False, debug=False)
    kvq_d = nc.dram_tensor("kvq", [tpc, P, kvq_cols], bf16,
                           kind="ExternalInput")
    io_d = nc.dram_tensor("iota", [P, ept], bf16, kind="ExternalInput")
    hm_d = nc.dram_tensor("hm", [P, H], bf16, kind="ExternalInput")
    out_d = nc.dram_tensor("out", [tpc // OPACK, P, OPACK * P], bf16,
                           kind="ExternalOutput")

    with TileContext(nc) as tc, ExitStack() as ctx:
        singles = ctx.enter_context(tc.tile_pool(name="singles", bufs=1))
        big = ctx.enter_context(tc.tile_pool(name="big", bufs=8))
        med = ctx.enter_context(tc.tile_pool(name="med", bufs=3))
        sml = ctx.enter_context(tc.tile_pool(name="sml", bufs=4))
        ps_sc = ctx.enter_context(tc.tile_pool(name="ps_sc", bufs=5, space="PSUM"))
        ps_ag = ctx.enter_context(tc.tile_pool(name="ps_ag", bufs=3, space="PSUM"))

        iota_wide_sb = singles.tile([P, ept], bf16)
        nc.sync.dma_start(out=iota_wide_sb[:], in_=io_d[:, :])
        hm_sb = singles.tile([P, H], bf16)
        nc.sync.dma_start(out=hm_sb[:], in_=hm_d[:, :])

        state = {}       # per-tile live tiles, keyed by tile index
        out4_box = [None]

        def front(t):
            """Stage A: needs only kvq(t) — dst map, one-hot, k*q, scores,
            exp. Emitted one tile ahead so DVE/ACT/PE stay fed while tile
            t-1 finishes its dependent tail."""
            kvq_sb = big.tile([P, kvq_cols], bf16, tag="kvq")
            nc.sync.dma_start(out=kvq_sb[:], in_=kvq_d[t])
            kt_sb = kvq_sb[:, 0:ept]
            qr_sb = kvq_sb[:, ept:ept + npair]
            dl_sb = kvq_sb[:, ept + npair + f_blocks * H * HS1:]

            # prodT[f, c] = kT[f, c] * qr[f, pair(c)]; parity-relabeled
            # columns make the pair broadcast a middle-dim stride-0 access
            # with a packed last dim (keeps DVE 2x). Emitted first so the
            # PE score burst can start while DVE builds the one-hot.
            prodT = med.tile([P, ept], bf16, tag="prodT", bufs=3)
            qr_b = qr_sb[:, :].to_broadcast([P, npair, 2]).transpose([0, 2, 1])
            nc.vector.tensor_tensor(
                out=prodT[:].rearrange("p (two j) -> p two j", two=2),
                in0=kt_sb[:].rearrange("p (two j) -> p two j", two=2),
                in1=qr_b,
                op=mybir.AluOpType.mult,
            )
            oh_en = med.tile([P, ept], bf16, tag="oh_en", bufs=4)
            dlw = med.tile([P, ept], bf16, tag="dlw", bufs=2)
            nc.scalar.copy(
                out=dlw[:].rearrange("p (b n) -> p b n", b=f_blocks),
                in_=dl_sb[:, :].to_broadcast([P, f_blocks, P]),
            )
            nc.vector.tensor_tensor(
                out=oh_en[:], in0=iota_wide_sb[:], in1=dlw[:],
                op=mybir.AluOpType.is_equal,
            )
            sc_ps = ps_sc.tile([P, f_blocks * H], f32, tag="sc")
            for b in range(f_blocks):
                nc.tensor.matmul(
                    out=sc_ps[:, b * H:(b + 1) * H],
                    lhsT=prodT[:, b * P:(b + 1) * P], rhs=hm_sb[:],
                    start=True, stop=True,
                )
            ex_t = sml.tile([P, f_blocks * H], bf16, tag="ex")
            nc.scalar.activation(
                out=ex_t[:], in_=sc_ps[:],
                func=mybir.ActivationFunctionType.Exp,
                scale=INV_SQRT_NF,
            )
            state[t] = (kvq_sb, oh_en, ex_t)

        def back(t):
            """Stage B: [ex | ex*v] and the scatter-add burst for tile t."""
            kvq_sb, oh_en, ex_t = state[t]
            v_sb = kvq_sb[:, ept + npair:ept + npair + f_blocks * H * HS1]
            agg_ps = ps_ag.tile([P, H * HS1], f32, tag="agg")
            # v is shipped stride-major [e, (s, b, h)] so the per-(b,h) ex
            # broadcast sits on a middle dim with a packed last dim.
            evex = med.tile([P, HS1 * f_blocks * H], bf16, tag="evex", bufs=3)
            nc.vector.tensor_tensor(
                out=evex[:].rearrange("p (s x) -> p s x", s=HS1),
                in0=v_sb[:].rearrange("p (s x) -> p s x", s=HS1),
                in1=ex_t[:].to_broadcast(
                    [P, f_blocks * H, HS1]).transpose([0, 2, 1]),
                op=mybir.AluOpType.mult,
            )
            evex_sv = evex[:].rearrange("p (s b h) -> p s b h",
                                        s=HS1, b=f_blocks)
            for b in range(f_blocks):
                nc.tensor.matmul(
                    out=agg_ps[:],
                    lhsT=oh_en[:, b * P:(b + 1) * P],
                    rhs=evex_sv[:, :, b, :],
                    start=(b == 0), stop=(b == f_blocks - 1),
                )
            state[t] = agg_ps

        def norm(t):
            """Stage C: normalize tile t and write out in OPACK groups.
            (zero-degree nodes produce NaN here; the host zeroes them)"""
            agg_ps = state.pop(t)
            agg_v = agg_ps[:].rearrange("p (s h) -> p s h", s=HS1)
            inv = sml.tile([P, H], f32, tag="inv")
            nc.vector.reciprocal(out=inv[:], in_=agg_v[:, 0, :])
            if t % OPACK == 0:
                out4_box[0] = med.tile([P, OPACK * P], bf16, tag="out4",
                                       bufs=2, name="out4_sb")
            out4_sb = out4_box[0]
            nc.vector.tensor_tensor(
                out=out4_sb[:, (t % OPACK) * P:(t % OPACK + 1) * P].rearrange(
                    "p (s h) -> p s h", s=HS),
                in0=agg_v[:, 1:, :],
                in1=inv[:].to_broadcast([P, H, HS]).transpose([0, 2, 1]),
                op=mybir.AluOpType.mult,
            )
            if t % OPACK == OPACK - 1:
                nc.sync.dma_start(out=out_d[t // OPACK], in_=out4_sb[:])

        # norm(t-1) sits between front(t) and back(t): Tile's cross-engine
        # wait thresholds are program-order conservative, so this makes the
        # reciprocal wait only for the (cheap, early) score burst of tile t
        # instead of the scatter burst.
        for t in range(tpc + 1):
            if t < tpc:
                front(t)
            if t >= 1:
                norm(t - 1)
            if t < tpc:
                back(t)
    nc.compile()
    return nc


# ------------------------------------------------------------ host plumbing
def _build_tiles(cum2, n_nodes, ept):
    """Greedy cut of nodes into (<=128 nodes, <=ept padded slots) tiles."""
    tiles = []
    n0 = 0
    while n0 < n_nodes:
        n1 = int(np.searchsorted(cum2, cum2[n0] + ept, side="right")) - 1
        n1 = min(n1, n0 + P, n_nodes)
        if n1 <= n0:
            raise ValueError(f"node {n0} has padded degree > {ept}; unsupported")
        tiles.append((n0, n1))
        n0 = n1
    return tiles


def _prep_inputs(value, key, query_0, query_1, edge_index,
                 tpc=T_PC, f_blocks=F_BLOCKS, n_cores=N_CORES, v_bf16=True):
    """Sort/tile/pad on the host; returns per-core input maps + assembly info."""
    import ml_dtypes
    bf16 = ml_dtypes.bfloat16

    ept = f_blocks * P
    npair = ept // 2
    value = np.ascontiguousarray(np.asarray(value, dtype=np.float32))
    key = np.ascontiguousarray(np.asarray(key, dtype=np.float32))
    q0 = np.asarray(query_0, dtype=np.float32)
    q1 = np.asarray(query_1, dtype=np.float32)
    ei = np.asarray(edge_index)
    n_nodes = q0.shape[0]
    n_edges = key.shape[0]

    dst = ei[1].astype(np.int64).ravel()
    order = np.argsort(dst, kind="stable")
    dsts = dst[order]
    counts = np.bincount(dsts, minlength=n_nodes)
    cum = np.zeros(n_nodes + 1, np.int64)
    cum[1:] = np.cumsum(counts)
    counts2 = (counts + 1) // 2 * 2          # per-node even padding
    cum2 = np.zeros(n_nodes + 1, np.int64)
    cum2[1:] = np.cumsum(counts2)

    tiles = _build_tiles(cum2, n_nodes, ept)
    t_total = len(tiles)
    if t_total > n_cores * tpc:
        raise ValueError(f"{t_total} tiles > capacity {n_cores * tpc}")
    q_per_core = (t_total + n_cores - 1) // n_cores  # real tiles per core
    t8 = n_cores * tpc

    node_tile = np.zeros(n_nodes, np.int64)   # global tile idx per node
    node_base = np.zeros(n_nodes, np.int64)   # slot offset of node in tile
    node_n0 = np.zeros(n_nodes, np.int64)     # first node of the tile
    tile_info = []  # (global_tile_idx, n0, n_cnt)
    tile_n0 = np.zeros(t8, np.int64)
    for i, (n0, n1) in enumerate(tiles):
        c, j = divmod(i, q_per_core)
        idx = c * tpc + j
        node_tile[n0:n1] = idx
        node_base[n0:n1] = cum2[n0:n1] - cum2[n0]
        node_n0[n0:n1] = n0
        tile_n0[idx] = n0
        tile_info.append((idx, n0, n1 - n0))

    # slot of each sorted edge (tile-local, node runs padded to even), then
    # parity relabel: col = (slot % 2) * npair + slot // 2
    erank = np.arange(n_edges, dtype=np.int64) - cum[dsts]
    eslot = node_base[dsts] + erank
    ecol = (eslot % 2) * npair + eslot // 2
    etile = node_tile[dsts]
    gcol = etile * ept + ecol                 # global column id

    slot_edge = np.zeros(t8 * ept, np.int64)
    slot_valid = np.zeros(t8 * ept, bool)
    dl = np.full(t8 * ept, PAD_DST, np.float32)
    slot_edge[gcol] = order
    slot_valid[gcol] = True
    dl[gcol] = (dsts - node_n0[dsts]).astype(np.float32)

    # pair index per tile: local node id of each slot pair (0 if empty)
    gpair = etile * npair + eslot // 2
    pidx = np.zeros(t8 * npair, np.int64)
    pidx[gpair] = dsts - node_n0[dsts]
    # qrun2[t, f, j] = q_cat[tile_n0[t] + pidx[t, j], f]
    q_cat = np.concatenate([q0, q1], axis=-1).reshape(n_nodes, NF)
    node_of_pair = np.minimum(
        tile_n0.repeat(npair) + pidx, n_nodes - 1)
    qr = np.ascontiguousarray(
        q_cat[node_of_pair].reshape(t8, npair, NF).transpose(0, 2, 1)
    ).astype(bf16)

    flat_edge = slot_edge
    flat_valid = slot_valid

    kf = key.reshape(n_edges, NF)
    k_slots = kf[flat_edge]
    k_slots[~flat_valid] = 0.0
    # kT: [t, f, col] (parity-relabeled columns), bf16
    kt = np.ascontiguousarray(k_slots.reshape(
        t8, ept, NF).transpose(0, 2, 1)).astype(bf16)
    del k_slots

    vf = value.reshape(n_edges, NF)
    v_slots = vf[flat_edge]
    v_slots[~flat_valid] = 0.0
    # stride-major v17: [t, e, s(17), b, h] with a leading all-1.0 s=0
    # plane (the ssum column), then the per-block dst columns: [t, e, b]
    v17 = np.empty((t8, P, HS1, f_blocks, H), np.float32)
    v17[:, :, 0] = 1.0
    v17[:, :, 1:] = v_slots.reshape(t8, f_blocks, P, H, HS).transpose(
        0, 2, 4, 1, 3)
    del v_slots
    vd = np.empty((t8, P, f_blocks * (H * HS1 + 1)), np.float32)
    vd[:, :, :f_blocks * H * HS1] = v17.reshape(t8, P, f_blocks * H * HS1)
    del v17
    vd[:, :, f_blocks * H * HS1:] = dl.reshape(
        t8, f_blocks, P).transpose(0, 2, 1)
    vd = vd.astype(bf16) if v_bf16 else vd

    kvq = np.concatenate([kt, qr, vd], axis=2)
    del kt, qr, vd

    iota = np.broadcast_to(np.arange(P, dtype=np.float32)[None, None, :],
                           (P, F_BLOCKS, P)).reshape(P, F_BLOCKS * P).astype(bf16)
    hm = np.zeros((NF, H), np.float32)
    for h in range(H):
        hm[h * HS:(h + 1) * HS, h] = 1.0
    hm = hm.astype(bf16)

    in_maps = []
    for c in range(n_cores):
        s = slice(c * tpc, (c + 1) * tpc)
        in_maps.append({
            "kvq": kvq[s], "iota": iota, "hm": hm,
        })
    zero_deg = np.flatnonzero(counts == 0)
    return in_maps, tile_info, n_nodes, zero_deg


# device out column c = s'*8 + h maps to feature f = h*16 + s'
_OUT_PERM = np.array([(c % H) * HS + c // H for c in range(NF)])


def _assemble(results, tile_info, n_nodes, zero_deg, tpc=T_PC):
    out = np.zeros((n_nodes, NF), np.float32)
    per_core = []
    for c in range(N_CORES):
        o4 = np.asarray(results[c]["out"], dtype=np.float32)
        per_core.append(o4.reshape(tpc // OPACK, P, OPACK, P)
                        .transpose(0, 2, 1, 3).reshape(tpc, P, P))
    for idx, n0, cnt in tile_info:
        c, j = divmod(idx, tpc)
        out[n0:n0 + cnt, _OUT_PERM] = per_core[c][j, :cnt]
    if zero_deg.size:
        out[zero_deg] = 0.0
    return out.reshape(n_nodes, NF // 4, 4)


def _get_nc(tpc=T_PC, f_blocks=F_BLOCKS, v_bf16=True):
    key = (tpc, f_blocks, v_bf16)
    if key not in _CACHE:
        _CACHE[key] = build_nc(tpc, f_blocks, v_bf16)
    return _CACHE[key]


def _needed_tpc(edge_index, n_nodes, ept, n_cores=N_CORES):
    dst = np.asarray(edge_index)[1].astype(np.int64).ravel()
    counts = np.bincount(dst, minlength=n_nodes)
    counts2 = (counts + 1) // 2 * 2
    cum2 = np.zeros(n_nodes + 1, np.int64)
    cum2[1:] = np.cumsum(counts2)
    t_total = len(_build_tiles(cum2, n_nodes, ept))
    tpc = (t_total + n_cores - 1) // n_cores
    return (tpc + OPACK - 1) // OPACK * OPACK  # out packs OPACK tiles per DMA


def _run(inputs, trace=False, tpc=T_PC, f_blocks=F_BLOCKS, v_bf16=True,
         **spmd_kwargs):
    global LAST_RESULTS
    from concourse.bass_utils import run_bass_kernel_spmd

    tpc = max(tpc, _needed_tpc(inputs["edge_index"],
                               np.asarray(inputs["query_0"]).shape[0],
                               f_blocks * P))
    nc = _get_nc(tpc, f_blocks, v_bf16)
    in_maps, tile_info, n_nodes, zero_deg = _prep_inputs(
        inputs["value"], inputs["key"], inputs["query_0"], inputs["query_1"],
        inputs["edge_index"], tpc=tpc, f_blocks=f_blocks, v_bf16=v_bf16)
    res = run_bass_kernel_spmd(
        nc, in_maps, list(range(N_CORES)), trace=trace, **spmd_kwargs)
    LAST_RESULTS = res
    return _assemble(res.results, tile_info, n_nodes, zero_deg, tpc=tpc)


def kernel(value, key, query_0, query_1, edge_index):
    return _run({
        "value": value, "key": key, "query_0": query_0,
        "query_1": query_1, "edge_index": edge_index,
    })


# revision 24
# speedup vs baseline: 1.0166x; 1.0166x over previous
"""SE(3) attention block (GNN message passing) on 8 Trainium2 NeuronCores.

Strategy
--------
Edges are sorted by destination node on the host. Nodes are cut into tiles of
(<=128 nodes, <=2048 edge slots); each node's edge run is padded to an even
number of slots so slot PAIRS always share one dst node. Tiles are distributed
contiguously across the 8 cores, so every (node, head) softmax group lives
entirely on one core and inside one tile -> no cross-device collectives.

The destination-node query is shipped ONCE PER SLOT-PAIR (qrun2, [128 feat,
1024 pairs] per tile) instead of once per slot, halving what used to be the
largest input stream. On device the pair query is consumed directly by the
k*q product through a stride-0 broadcast access pattern: edge slots are
relabeled so all even slots occupy columns 0..1023 and odd slots 1024..2047
(host-side permutation, free), which turns the pair broadcast into a
middle-dim broadcast [128, 2, 1024] whose last dim stays packed -> DVE keeps
its 2x (16-bit) throughput for the product.

Per node tile the device kernel:
  1. widens the per-block dst map to a dense [128, 2048] map (one ACT op)
     and builds the one-hot edge->local-node matrix with a single 2x DVE
     is_equal against a premade iota,
  2. prodT = kT * q_pair broadcast (one 2x DVE op),
  3. per-head scores via 16 head-mask matmuls (N=8) into one PSUM bank,
  4. one exp over the tile's [128, 128] scores (ACT, bf16 out),
  5. widens ex to the interleaved 17-stride (ACT) and forms the
     [ex | ex*v] scatter rhs with one whole-tile DVE 2x multiply
     (v is sent from the host with a 1.0 column per head: 17 cols/head),
  6. 16 back-to-back scatter-add matmuls (bf16, N=136) accumulate into a
     [128, 136] PSUM tile.
The tile is then normalized by 1/sum(exp) (strided APs pull ssum/agg out of
the interleaved accumulator) and written out in bf16. The host scatters
per-tile rows back into the full [N, 32, 4] f32 output. GPSIMD is left idle:
TRN2's Pool engine rejects general tensor ops (ISA check) and its custom
gather library costs ~29us fixed per invocation.
"""

import math
import numpy as np

# ---------------------------------------------------------------- constants
N_CORES = 8
P = 128                 # partitions / nodes per tile / edges per block
F_BLOCKS = 16           # edge blocks per node tile
EPT = F_BLOCKS * P      # edge slots per tile (2048)
NPAIR = EPT // 2
T_PC = 51               # node tiles per core (multiple of OPACK)
OPACK = 3               # output tiles packed per DMA
H = 8                   # heads
NF = 128                # features per edge (32*4)
HS = NF // H            # head size (16)
HS1 = HS + 1            # interleaved head stride (ex + 16 features)
N_NODES = 50000
E_EDGES = 800000
PAD_DST = 300.0         # local-dst sentinel for padding edge slots
INV_SQRT_NF = 1.0 / math.sqrt(NF)

_CACHE = {}
LAST_RESULTS = None     # BassKernelResults of the most recent run (for test.py)


# ---------------------------------------------------------------- device IR
def build_nc(tpc=T_PC, f_blocks=F_BLOCKS, v_bf16=True):
    """Build the per-core Bass/Tile program (identical on all 8 cores)."""
    from contextlib import ExitStack

    import concourse.bacc as bacc
    import concourse.mybir as mybir
    from concourse.tile import TileContext

    f32 = mybir.dt.float32
    bf16 = mybir.dt.bfloat16
    vdt = bf16 if v_bf16 else f32
    ept = f_blocks * P
    npair = ept // 2

    vcols = f_blocks * H * HS + f_blocks      # v16 + dl columns (2064)
    kvq_cols = ept + npair + vcols            # kt | qr | v16+dl

    nc = bacc.Bacc("TRN2", target_bir_lowering=False, debug=False)
    kvq_d = nc.dram_tensor("kvq", [tpc, P, kvq_cols], bf16,
                           kind="ExternalInput")
    io_d = nc.dram_tensor("iota", [P, ept], bf16, kind="ExternalInput")
    hm_d = nc.dram_tensor("hm", [P, H], bf16, kind="ExternalInput")
    out_d = nc.dram_tensor("out", [tpc // OPACK, P, OPACK * P], bf16,
                           kind="ExternalOutput")

    with TileContext(nc) as tc, ExitStack() as ctx:
        singles = ctx.enter_context(tc.tile_pool(name="singles", bufs=1))
        big = ctx.enter_context(tc.tile_pool(name="big", bufs=10))
        med = ctx.enter_context(tc.tile_pool(name="med", bufs=3))
        sml = ctx.enter_context(tc.tile_pool(name="sml", bufs=4))
        ps_sc = ctx.enter_context(tc.tile_pool(name="ps_sc", bufs=5, space="PSUM"))
        ps_ag = ctx.enter_context(tc.tile_pool(name="ps_ag", bufs=3, space="PSUM"))

        iota_wide_sb = singles.tile([P, ept], bf16)
        nc.sync.dma_start(out=iota_wide_sb[:], in_=io_d[:, :])
        hm_sb = singles.tile([P, H], bf16)
        nc.sync.dma_start(out=hm_sb[:], in_=hm_d[:, :])

        state = {}       # per-tile live tiles, keyed by tile index
        out4_box = [None]

        def front(t):
            """Stage A: needs only kvq(t) — dst map, one-hot, k*q, scores,
            exp. Emitted one tile ahead so DVE/ACT/PE stay fed while tile
            t-1 finishes its dependent tail."""
            kvq_sb = big.tile([P, kvq_cols], bf16, tag="kvq")
            nc.sync.dma_start(out=kvq_sb[:], in_=kvq_d[t])
            kt_sb = kvq_sb[:, 0:ept]
            qr_sb = kvq_sb[:, ept:ept + npair]
            dl_sb = kvq_sb[:, ept + npair + f_blocks * H * HS:]

            # prodT[f, c] = kT[f, c] * qr[f, pair(c)]; parity-relabeled
            # columns make the pair broadcast a middle-dim stride-0 access
            # with a packed last dim (keeps DVE 2x). Emitted first so the
            # PE score burst can start while DVE builds the one-hot.
            prodT = med.tile([P, ept], bf16, tag="prodT", bufs=3)
            qr_b = qr_sb[:, :].to_broadcast([P, npair, 2]).transpose([0, 2, 1])
            nc.vector.tensor_tensor(
                out=prodT[:].rearrange("p (two j) -> p two j", two=2),
                in0=kt_sb[:].rearrange("p (two j) -> p two j", two=2),
                in1=qr_b,
                op=mybir.AluOpType.mult,
            )
            oh_en = med.tile([P, ept], bf16, tag="oh_en", bufs=4)
            dlw = med.tile([P, ept], bf16, tag="dlw", bufs=2)
            nc.scalar.copy(
                out=dlw[:].rearrange("p (b n) -> p b n", b=f_blocks),
                in_=dl_sb[:, :].to_broadcast([P, f_blocks, P]),
            )
            nc.vector.tensor_tensor(
                out=oh_en[:], in0=iota_wide_sb[:], in1=dlw[:],
                op=mybir.AluOpType.is_equal,
            )
            sc_ps = ps_sc.tile([P, f_blocks * H], f32, tag="sc")
            for b in range(f_blocks):
                nc.tensor.matmul(
                    out=sc_ps[:, b * H:(b + 1) * H],
                    lhsT=prodT[:, b * P:(b + 1) * P], rhs=hm_sb[:],
                    start=True, stop=True,
                )
            ex_t = sml.tile([P, f_blocks * H], bf16, tag="ex")
            nc.scalar.activation(
                out=ex_t[:], in_=sc_ps[:],
                func=mybir.ActivationFunctionType.Exp,
                scale=INV_SQRT_NF,
            )
            state[t] = (kvq_sb, oh_en, ex_t)

        def back(t):
            """Stage B: [ex | ex*v] and the scatter-add burst for tile t."""
            kvq_sb, oh_en, ex_t = state[t]
            v_sb = kvq_sb[:, ept + npair:ept + npair + f_blocks * H * HS]
            agg_ps = ps_ag.tile([P, H * HS1], f32, tag="agg")
            # v is shipped stride-major [e, (s, b, h)] so the per-(b,h) ex
            # broadcast sits on a middle dim with a packed last dim. The
            # s=0 (ssum) plane is ex itself, copied in by ACT.
            evex = med.tile([P, HS1 * f_blocks * H], bf16, tag="evex", bufs=3)
            evex_s = evex[:].rearrange("p (s x) -> p s x", s=HS1)
            nc.scalar.copy(out=evex_s[:, 0, :], in_=ex_t[:])
            nc.vector.tensor_tensor(
                out=evex_s[:, 1:, :],
                in0=v_sb[:].rearrange("p (s x) -> p s x", s=HS),
                in1=ex_t[:].to_broadcast(
                    [P, f_blocks * H, HS]).transpose([0, 2, 1]),
                op=mybir.AluOpType.mult,
            )
            evex_sv = evex[:].rearrange("p (s b h) -> p s b h",
                                        s=HS1, b=f_blocks)
            for b in range(f_blocks):
                nc.tensor.matmul(
                    out=agg_ps[:],
                    lhsT=oh_en[:, b * P:(b + 1) * P],
                    rhs=evex_sv[:, :, b, :],
                    start=(b == 0), stop=(b == f_blocks - 1),
                )
            state[t] = agg_ps

        def norm(t):
            """Stage C: normalize tile t and write out in OPACK groups.
            (zero-degree nodes produce NaN here; the host zeroes them)"""
            agg_ps = state.pop(t)
            agg_v = agg_ps[:].rearrange("p (s h) -> p s h", s=HS1)
            inv = sml.tile([P, H], f32, tag="inv")
            nc.vector.reciprocal(out=inv[:], in_=agg_v[:, 0, :])
            if t % OPACK == 0:
                out4_box[0] = med.tile([P, OPACK * P], bf16, tag="out4",
                                       bufs=2, name="out4_sb")
            out4_sb = out4_box[0]
            nc.vector.tensor_tensor(
                out=out4_sb[:, (t % OPACK) * P:(t % OPACK + 1) * P].rearrange(
                    "p (s h) -> p s h", s=HS),
                in0=agg_v[:, 1:, :],
                in1=inv[:].to_broadcast([P, H, HS]).transpose([0, 2, 1]),
                op=mybir.AluOpType.mult,
            )
            if t % OPACK == OPACK - 1:
                nc.sync.dma_start(out=out_d[t // OPACK], in_=out4_sb[:])

        # norm(t-1) sits between front(t) and back(t): Tile's cross-engine
        # wait thresholds are program-order conservative, so this makes the
        # reciprocal wait only for the (cheap, early) score burst of tile t
        # instead of the scatter burst.
        for t in range(tpc + 1):
            if t < tpc:
                front(t)
            if t >= 1:
                norm(t - 1)
            if t < tpc:
                back(t)
    nc.compile()
    return nc


# ------------------------------------------------------------ host plumbing
def _build_tiles(cum2, n_nodes, ept):
    """Greedy cut of nodes into (<=128 nodes, <=ept padded slots) tiles."""
    tiles = []
    n0 = 0
    while n0 < n_nodes:
        n1 = int(np.searchsorted(cum2, cum2[n0] + ept, side="right")) - 1
        n1 = min(n1, n0 + P, n_nodes)
        if n1 <= n0:
            raise ValueError(f"node {n0} has padded degree > {ept}; unsupported")
        tiles.append((n0, n1))
        n0 = n1
    return tiles


def _prep_inputs(value, key, query_0, query_1, edge_index,
                 tpc=T_PC, f_blocks=F_BLOCKS, n_cores=N_CORES, v_bf16=True):
    """Sort/tile/pad on the host; returns per-core input maps + assembly info."""
    import ml_dtypes
    bf16 = ml_dtypes.bfloat16

    ept = f_blocks * P
    npair = ept // 2
    value = np.ascontiguousarray(np.asarray(value, dtype=np.float32))
    key = np.ascontiguousarray(np.asarray(key, dtype=np.float32))
    q0 = np.asarray(query_0, dtype=np.float32)
    q1 = np.asarray(query_1, dtype=np.float32)
    ei = np.asarray(edge_index)
    n_nodes = q0.shape[0]
    n_edges = key.shape[0]

    dst = ei[1].astype(np.int64).ravel()
    order = np.argsort(dst, kind="stable")
    dsts = dst[order]
    counts = np.bincount(dsts, minlength=n_nodes)
    cum = np.zeros(n_nodes + 1, np.int64)
    cum[1:] = np.cumsum(counts)
    counts2 = (counts + 1) // 2 * 2          # per-node even padding
    cum2 = np.zeros(n_nodes + 1, np.int64)
    cum2[1:] = np.cumsum(counts2)

    tiles = _build_tiles(cum2, n_nodes, ept)
    t_total = len(tiles)
    if t_total > n_cores * tpc:
        raise ValueError(f"{t_total} tiles > capacity {n_cores * tpc}")
    q_per_core = (t_total + n_cores - 1) // n_cores  # real tiles per core
    t8 = n_cores * tpc

    node_tile = np.zeros(n_nodes, np.int64)   # global tile idx per node
    node_base = np.zeros(n_nodes, np.int64)   # slot offset of node in tile
    node_n0 = np.zeros(n_nodes, np.int64)     # first node of the tile
    tile_info = []  # (global_tile_idx, n0, n_cnt)
    tile_n0 = np.zeros(t8, np.int64)
    for i, (n0, n1) in enumerate(tiles):
        c, j = divmod(i, q_per_core)
        idx = c * tpc + j
        node_tile[n0:n1] = idx
        node_base[n0:n1] = cum2[n0:n1] - cum2[n0]
        node_n0[n0:n1] = n0
        tile_n0[idx] = n0
        tile_info.append((idx, n0, n1 - n0))

    # slot of each sorted edge (tile-local, node runs padded to even), then
    # parity relabel: col = (slot % 2) * npair + slot // 2
    erank = np.arange(n_edges, dtype=np.int64) - cum[dsts]
    eslot = node_base[dsts] + erank
    ecol = (eslot % 2) * npair + eslot // 2
    etile = node_tile[dsts]
    gcol = etile * ept + ecol                 # global column id

    slot_edge = np.zeros(t8 * ept, np.int64)
    slot_valid = np.zeros(t8 * ept, bool)
    dl = np.full(t8 * ept, PAD_DST, np.float32)
    slot_edge[gcol] = order
    slot_valid[gcol] = True
    dl[gcol] = (dsts - node_n0[dsts]).astype(np.float32)

    # pair index per tile: local node id of each slot pair (0 if empty)
    gpair = etile * npair + eslot // 2
    pidx = np.zeros(t8 * npair, np.int64)
    pidx[gpair] = dsts - node_n0[dsts]
    # qrun2[t, f, j] = q_cat[tile_n0[t] + pidx[t, j], f]
    q_cat = np.concatenate([q0, q1], axis=-1).reshape(n_nodes, NF)
    node_of_pair = np.minimum(
        tile_n0.repeat(npair) + pidx, n_nodes - 1)
    qr = np.ascontiguousarray(
        q_cat[node_of_pair].reshape(t8, npair, NF).transpose(0, 2, 1)
    ).astype(bf16)

    flat_edge = slot_edge
    flat_valid = slot_valid

    kf = key.reshape(n_edges, NF)
    k_slots = kf[flat_edge]
    k_slots[~flat_valid] = 0.0
    # kT: [t, f, col] (parity-relabeled columns), bf16
    kt = np.ascontiguousarray(k_slots.reshape(
        t8, ept, NF).transpose(0, 2, 1)).astype(bf16)
    del k_slots

    vf = value.reshape(n_edges, NF)
    v_slots = vf[flat_edge]
    v_slots[~flat_valid] = 0.0
    # stride-major v16: [t, e, s(16), b, h] (the device prepends the ex
    # plane itself), then the per-block dst columns: [t, e, b]
    vd = np.empty((t8, P, f_blocks * H * HS + f_blocks), np.float32)
    vd[:, :, :f_blocks * H * HS] = v_slots.reshape(
        t8, f_blocks, P, H, HS).transpose(0, 2, 4, 1, 3).reshape(
        t8, P, f_blocks * H * HS)
    del v_slots
    vd[:, :, f_blocks * H * HS:] = dl.reshape(
        t8, f_blocks, P).transpose(0, 2, 1)
    vd = vd.astype(bf16) if v_bf16 else vd

    kvq = np.concatenate([kt, qr, vd], axis=2)
    del kt, qr, vd

    iota = np.broadcast_to(np.arange(P, dtype=np.float32)[None, None, :],
                           (P, F_BLOCKS, P)).reshape(P, F_BLOCKS * P).astype(bf16)
    hm = np.zeros((NF, H), np.float32)
    for h in range(H):
        hm[h * HS:(h + 1) * HS, h] = 1.0
    hm = hm.astype(bf16)

    in_maps = []
    for c in range(n_cores):
        s = slice(c * tpc, (c + 1) * tpc)
        in_maps.append({
            "kvq": kvq[s], "iota": iota, "hm": hm,
        })
    zero_deg = np.flatnonzero(counts == 0)
    return in_maps, tile_info, n_nodes, zero_deg


# device out column c = s'*8 + h maps to feature f = h*16 + s'
_OUT_PERM = np.array([(c % H) * HS + c // H for c in range(NF)])


def _assemble(results, tile_info, n_nodes, zero_deg, tpc=T_PC):
    out = np.zeros((n_nodes, NF), np.float32)
    per_core = []
    for c in range(N_CORES):
        o4 = np.asarray(results[c]["out"], dtype=np.float32)
        per_core.append(o4.reshape(tpc // OPACK, P, OPACK, P)
                        .transpose(0, 2, 1, 3).reshape(tpc, P, P))
    for idx, n0, cnt in tile_info:
        c, j = divmod(idx, tpc)
        out[n0:n0 + cnt, _OUT_PERM] = per_core[c][j, :cnt]
    if zero_deg.size:
        out[zero_deg] = 0.0
    return out.reshape(n_nodes, NF // 4, 4)


def _get_nc(tpc=T_PC, f_blocks=F_BLOCKS, v_bf16=True):
    key = (tpc, f_blocks, v_bf16)
    if key not in _CACHE:
        _CACHE[key] = build_nc(tpc, f_blocks, v_bf16)
    return _CACHE[key]


def _needed_tpc(edge_index, n_nodes, ept, n_cores=N_CORES):
    dst = np.asarray(edge_index)[1].astype(np.int64).ravel()
    counts = np.bincount(dst, minlength=n_nodes)
    counts2 = (counts + 1) // 2 * 2
    cum2 = np.zeros(n_nodes + 1, np.int64)
    cum2[1:] = np.cumsum(counts2)
    t_total = len(_build_tiles(cum2, n_nodes, ept))
    tpc = (t_total + n_cores - 1) // n_cores
    return (tpc + OPACK - 1) // OPACK * OPACK  # out packs OPACK tiles per DMA


def _run(inputs, trace=False, tpc=T_PC, f_blocks=F_BLOCKS, v_bf16=True,
         **spmd_kwargs):
    global LAST_RESULTS
    from concourse.bass_utils import run_bass_kernel_spmd

    tpc = max(tpc, _needed_tpc(inputs["edge_index"],
                               np.asarray(inputs["query_0"]).shape[0],
                               f_blocks * P))
    nc = _get_nc(tpc, f_blocks, v_bf16)
    in_maps, tile_info, n_nodes, zero_deg = _prep_inputs(
        inputs["value"], inputs["key"], inputs["query_0"], inputs["query_1"],
        inputs["edge_index"], tpc=tpc, f_blocks=f_blocks, v_bf16=v_bf16)
    res = run_bass_kernel_spmd(
        nc, in_maps, list(range(N_CORES)), trace=trace, **spmd_kwargs)
    LAST_RESULTS = res
    return _assemble(res.results, tile_info, n_nodes, zero_deg, tpc=tpc)


def kernel(value, key, query_0, query_1, edge_index):
    return _run({
        "value": value, "key": key, "query_0": query_0,
        "query_1": query_1, "edge_index": edge_index,
    })
